# revision 18
# baseline (speedup 1.0000x reference)
"""AdaptiveTrendExtractor Trainium2 kernel (8-core data parallel).

Math (per row r of x reshaped to (B*N, L)):
  f_s = conv1d(x_r, w_s, 'same')            s in 4 scales (3,5,7,9), NO bias
  (softmax/entropy are invariant to the conv bias, so biases are folded
   into the final output as dot(weights, b) instead)
  Z_s   = sum_l exp(f_s)                     (no max-subtraction; |f| small)
  ent_s = log Z_s - (sum_l f_s*exp(f_s))/Z_s
  h     = relu((-ent) @ (-W1) + b1)          (we compute negent = -ent)
  wts   = softmax(h @ W2 + b2)
  out_r = sum_s wts_s * f_s + dot(wts, b)
"""

import numpy as np
import ml_dtypes

import concourse.bass as bass
import concourse.mybir as mybir
import concourse.tile as tile_mod
from concourse.tile import TileContext
from concourse.bass_utils import run_bass_kernel_spmd

F32 = mybir.dt.float32
BF16 = mybir.dt.bfloat16
AX = mybir.AxisListType
OP = mybir.AluOpType
AF = mybir.ActivationFunctionType

B, N, L = 64, 321, 720
NCORES = 8
R = B * N // NCORES          # 2568 rows per core
P = 128                      # partitions / rows per tile
NT = (R + P - 1) // P        # 21 tiles (last overlaps)
NC_CHUNK = 6                 # L chunks of 120 outputs each
CW = 120                     # outputs per chunk
PAD = 4                      # max k//2
SCALES = [3, 5, 7, 9]
NS = len(SCALES)
BANDW = NS * CW              # 480

# ---------------------------------------------------------------------------
# Patch: this walrus build rejects >1 sem wait on the TileContext final Drain
# (TPB_CTRL "Too many sync wait commands"); split waits over several drains.
_ScopedClock = tile_mod.ScopedClock


def _patched_dab(self, tick_clock, wait_clock):
    import bass_rust as _br

    nc = self.nc
    drain_inst = nc.sync.drain()
    wait_clock.add_sem_waits(
        drain_inst.ins, _ScopedClock({None: tick_clock.global_clock})
    )
    waits = list(drain_inst.ins.sync_info.on_wait)
    if len(waits) > 1:
        si = drain_inst.ins.sync_info
        si.on_wait = waits[:1]
        drain_inst.ins.sync_info = si
        for w in waits[1:]:
            d2 = nc.sync.drain()
            d2.ins.sync_info = _br.SyncInfo(on_wait=[w], on_update=[])
    nc.all_engine_barrier()
    popped = nc._tile_sem_poison_stack.pop()
    assert popped is self._sem_poison
    nc.clear_and_free_semaphores(list(self.sems.allocated().values()))
    nc.all_engine_barrier()


TileContext._drain_and_barrier = _patched_dab


def _split_excess_waits(nc, maxw=1):
    """walrus in this env rejects >maxw sem-waits on one instruction;
    hoist excess waits onto same-engine NoOps inserted just before."""
    import bass_rust as _br

    fn = nc.m.functions[0]
    # (block_idx, pos, engine, [waits]) for every violation
    plans = []
    for bi, blk in enumerate(fn.blocks):
        for pi, ins in enumerate(blk.instructions):
            si = ins.sync_info
            if si is None or not si.on_wait:
                continue
            waits = list(si.on_wait)
            if len(waits) > maxw:
                plans.append((bi, pi, ins, waits))
    if not plans:
        return
    # create nops (they get appended to the current bb; we'll move them)
    nop_map = {}
    created = []
    for bi, pi, ins, waits in plans:
        eng = nc.engines[ins.engine]
        nops = []
        for w in waits[:-maxw]:
            n = eng.nop()
            n.ins.sync_info = _br.SyncInfo(on_wait=[w], on_update=[])
            nops.append(n.ins)
            created.append(n.ins)
        si = ins.sync_info
        si.on_wait = waits[-maxw:]
        ins.sync_info = si
        nop_map[ins.name] = nops
    created_names = {n.name for n in created}
    for blk in fn.blocks:
        newl = []
        for ins in blk.instructions:
            if ins.name in created_names:
                continue  # remove from wherever engine.nop() appended it
            if ins.name in nop_map:
                newl.extend(nop_map[ins.name])
            newl.append(ins)
        blk.instructions = newl
# ---------------------------------------------------------------------------


def build_nc():
    nc = bass.Bass()
    x = nc.declare_dram_parameter("x", [R, L], F32, isOutput=False)
    bands = nc.declare_dram_parameter("bands", [P, BANDW], BF16, isOutput=False)
    identb = nc.declare_dram_parameter("identb", [P, P], BF16, isOutput=False)
    identf = nc.declare_dram_parameter("identf", [P, P], F32, isOutput=False)
    w1aug = nc.declare_dram_parameter("w1aug", [5, 32], F32, isOutput=False)
    w2aug = nc.declare_dram_parameter("w2aug", [33, 4], F32, isOutput=False)
    bvec = nc.declare_dram_parameter("bvec", [P, 4], F32, isOutput=False)
    y = nc.declare_dram_parameter("out", [R, L], F32, isOutput=True)

    with TileContext(nc) as tc:
        with (
            tc.tile_pool(name="const", bufs=1) as constp,
            tc.tile_pool(name="xin", bufs=3) as xpool,
            tc.tile_pool(name="xbf", bufs=3) as xbpool,
            tc.tile_pool(name="xts", bufs=2) as xtpool,
            tc.tile_pool(name="ebuf", bufs=2) as epool,
            tc.tile_pool(name="fbuf", bufs=3) as fbpool,
            tc.tile_pool(name="scr", bufs=2) as scrpool,
            tc.tile_pool(name="accp", bufs=4) as accpool,
            tc.tile_pool(name="youtp", bufs=3) as ypool,
            tc.tile_pool(name="small", bufs=8) as small,
            tc.tile_pool(name="fps", bufs=1, space="PSUM") as fpool,
            tc.tile_pool(name="mps", bufs=2, space="PSUM") as mpool,
        ):
            bands_t = constp.tile([P, BANDW], BF16)
            nc.sync.dma_start(out=bands_t[:], in_=bands[:])
            identb_t = constp.tile([P, P], BF16)
            nc.sync.dma_start(out=identb_t[:], in_=identb[:])
            identf_t = constp.tile([P, P], F32)
            nc.sync.dma_start(out=identf_t[:], in_=identf[:])
            w1_t = constp.tile([5, 32], F32)
            nc.sync.dma_start(out=w1_t[:], in_=w1aug[:])
            w2_t = constp.tile([33, 4], F32)
            nc.sync.dma_start(out=w2_t[:], in_=w2aug[:])
            bvec_t = constp.tile([P, 4], F32)
            nc.sync.dma_start(out=bvec_t[:], in_=bvec[:])

            def stage_a(t):
                r0 = min(P * t, R - P)
                st = {"r0": r0}
                # -- load + pad + cast -----------------------------------
                xp = xpool.tile([P, L + 2 * PAD], F32)
                nc.sync.dma_start(out=xp[:, PAD : PAD + L], in_=x[r0 : r0 + P, :])
                nc.gpsimd.memset(xp[:, 0:PAD], 0.0)
                nc.gpsimd.memset(xp[:, PAD + L :], 0.0)
                xb = xbpool.tile([P, L + 2 * PAD], BF16)
                nc.gpsimd.tensor_copy(xb[:], xp[:])

                # -- transpose 6 chunks to PSUM, evacuate to SBUF --------
                xtp = mpool.tile([P, NC_CHUNK * P], BF16, tag="ps")
                for c in range(NC_CHUNK):
                    nc.tensor.transpose(
                        xtp[:, c * P : (c + 1) * P],
                        xb[:, c * CW : c * CW + P],
                        identb_t[:],
                    )
                xt = xtpool.tile([P, NC_CHUNK * P], BF16)
                nc.vector.tensor_copy(xt[:], xtp[:])

                # -- conv: banded matmuls, all 4 scales per chunk --------
                f = fpool.tile([P, NC_CHUNK * 512], F32)
                for c in range(NC_CHUNK):
                    nc.tensor.matmul(
                        f[:, c * 512 : c * 512 + BANDW],
                        lhsT=xt[:, c * P : (c + 1) * P],
                        rhs=bands_t[:],
                        start=True,
                        stop=True,
                    )
                f_cx = f.rearrange("p (c x) -> p c x", x=512)
                f_scj = f_cx[:, :, 0:BANDW].rearrange("p c (s j) -> p s c j", j=CW)

                # -- evacuate f to SBUF bf16 (the ONLY PSUM reader, so
                #    next tile's conv can start after just this ~2.9us) --
                fb = fbpool.tile([P, NS * L], BF16)
                fb_scj = fb.rearrange("p (s c j) -> p s c j", c=NC_CHUNK, j=CW)
                nc.scalar.activation(fb_scj, f_scj, AF.Copy)
                # -- exp per scale from fb; accum_out gives Z for free ---
                e = epool.tile([P, NS * L], BF16)
                Zt = small.tile([P, 4], F32)
                for s in range(NS):
                    nc.scalar.activation(
                        e[:, s * L : (s + 1) * L],
                        fb[:, s * L : (s + 1) * L],
                        AF.Exp,
                        accum_out=Zt[:, s : s + 1],
                    )
                st.update(e=e, fb=fb, Zt=Zt)
                return st

            def stage_b(st):
                r0, e, fb, Zt = st["r0"], st["e"], st["fb"], st["Zt"]
                lnZ = small.tile([P, 4], F32)
                nc.scalar.activation(lnZ[:], Zt[:], AF.Ln)
                rZ = small.tile([P, 4], F32)
                nc.vector.reciprocal(rZ[:], Zt[:])

                # -- sum f*e per scale (fused mult+reduce) ---------------
                sfe = small.tile([P, 4], F32)
                scr = scrpool.tile([P, L], BF16)
                for s in range(NS):
                    nc.vector.scalar_tensor_tensor(
                        out=scr[:],
                        in0=e[:, s * L : (s + 1) * L],
                        scalar=1.0,
                        in1=fb[:, s * L : (s + 1) * L],
                        op0=OP.mult,
                        op1=OP.mult,
                        accum_out=sfe[:, s : s + 1],
                    )

                # -- negent = sfe/Z - lnZ  (col 4 = ones for bias aug) ---
                ne = small.tile([P, 4], F32)
                nc.gpsimd.tensor_tensor(ne[:], sfe[:], rZ[:], OP.mult)
                ne2 = small.tile([P, 5], F32)
                nc.gpsimd.tensor_tensor(ne2[:, 0:4], ne[:], lnZ[:], OP.subtract)
                nc.gpsimd.memset(ne2[:, 4:5], 1.0)

                # -- MLP -------------------------------------------------
                entTp = mpool.tile([5, P], F32, tag="ps")
                nc.tensor.transpose(entTp[:], ne2[:], identf_t[:])
                entT = small.tile([5, P], F32)
                nc.vector.tensor_copy(entT[:], entTp[:])
                hp = mpool.tile([P, 32], F32, tag="ps")
                nc.tensor.matmul(hp[:], lhsT=entT[:], rhs=w1_t[:], start=True, stop=True)
                h = small.tile([P, 33], F32)
                nc.vector.tensor_scalar_max(h[:, 0:32], hp[:], 0.0)
                nc.gpsimd.memset(h[:, 32:33], 1.0)
                hTp = mpool.tile([33, P], F32, tag="ps")
                nc.tensor.transpose(hTp[:], h[:], identf_t[:])
                hT = small.tile([33, P], F32)
                nc.vector.tensor_copy(hT[:], hTp[:])
                lgp = mpool.tile([P, 4], F32, tag="ps")
                nc.tensor.matmul(lgp[:], lhsT=hT[:], rhs=w2_t[:], start=True, stop=True)

                elog = small.tile([P, 4], F32)
                Z4 = small.tile([P, 1], F32)
                nc.scalar.activation(elog[:], lgp[:], AF.Exp, accum_out=Z4[:])
                rZ4 = small.tile([P, 1], F32)
                nc.vector.reciprocal(rZ4[:], Z4[:])
                wts = small.tile([P, 4], F32)
                nc.gpsimd.tensor_scalar(wts[:], elog[:], rZ4[:], None, OP.mult)
                wscr = small.tile([P, 4], F32)
                bdot = small.tile([P, 1], F32)
                nc.vector.scalar_tensor_tensor(
                    out=wscr[:],
                    in0=wts[:],
                    scalar=1.0,
                    in1=bvec_t[:],
                    op0=OP.mult,
                    op1=OP.mult,
                    accum_out=bdot[:],
                )

                # -- trend = sum_s wts_s * f_s + bdot --------------------
                # 4 fast single-src products, then accumulate-DMAs (SWDGE)
                p0 = accpool.tile([P, L], BF16, tag="acc")
                p1 = accpool.tile([P, L], BF16, tag="acc")
                p2 = accpool.tile([P, L], BF16, tag="acc")
                p3 = accpool.tile([P, L], BF16, tag="acc")
                yt = ypool.tile([P, L], F32)
                nc.vector.tensor_scalar(
                    p0[:], fb[:, 0:L], wts[:, 0:1], bdot[:], OP.mult, OP.add
                )
                nc.vector.tensor_scalar(
                    p1[:], fb[:, L : 2 * L], wts[:, 1:2], None, OP.mult
                )
                nc.vector.tensor_scalar(
                    p2[:], fb[:, 2 * L : 3 * L], wts[:, 2:3], None, OP.mult
                )
                nc.vector.tensor_scalar(
                    p3[:], fb[:, 3 * L : 4 * L], wts[:, 3:4], None, OP.mult
                )
                nc.gpsimd.dma_start(out=p0[:], in_=p1[:], accum_op=OP.add)
                nc.gpsimd.dma_start(out=p0[:], in_=p2[:], accum_op=OP.add)
                nc.gpsimd.dma_start(out=p0[:], in_=p3[:], accum_op=OP.add)
                nc.vector.tensor_copy(yt[:], p0[:])
                nc.sync.dma_start(out=y[r0 : r0 + P, :], in_=yt[:])

            pend = {}
            for t in range(NT + 1):
                if t < NT:
                    pend[t] = stage_a(t)
                if t >= 1:
                    stage_b(pend.pop(t - 1))
    _split_excess_waits(nc)
    return nc


_NC = None


def _get_nc():
    global _NC
    if _NC is None:
        _NC = build_nc()
    return _NC


def _host_consts(cw, cb, W1, b1, W2, b2):
    bands = np.zeros((P, BANDW), np.float32)
    for s, (k, w) in enumerate(zip(SCALES, cw)):
        w = np.asarray(w, np.float32).reshape(-1)
        for lp in range(CW):
            for j in range(k):
                kidx = lp + j + PAD - k // 2
                bands[kidx, s * CW + lp] = w[j]
    consts = {
        "bands": bands.astype(ml_dtypes.bfloat16),
        "identb": np.eye(P, dtype=ml_dtypes.bfloat16),
        "identf": np.eye(P, dtype=np.float32),
        "w1aug": np.concatenate(
            [-np.asarray(W1, np.float32), np.asarray(b1, np.float32)[None, :]], 0
        ),
        "w2aug": np.concatenate(
            [np.asarray(W2, np.float32), np.asarray(b2, np.float32)[None, :]], 0
        ),
        "bvec": np.tile(
            np.asarray(cb, np.float32).reshape(1, 4), (P, 1)
        ).astype(np.float32),
    }
    return consts


def run(inputs, **spmd_kwargs):
    nc = _get_nc()
    x = np.asarray(inputs["x"], np.float32).reshape(B * N, L)
    consts = _host_consts(
        [inputs[f"cw{i}"] for i in range(4)],
        [np.asarray(inputs[f"cb{i}"], np.float32).reshape(()) for i in range(4)],
        inputs["W1"],
        inputs["b1"],
        inputs["W2"],
        inputs["b2"],
    )
    in_maps = []
    for i in range(NCORES):
        m = {"x": np.ascontiguousarray(x[i * R : (i + 1) * R])}
        m.update(consts)
        in_maps.append(m)
    res = run_bass_kernel_spmd(nc, in_maps, core_ids=list(range(NCORES)), **spmd_kwargs)
    y = np.concatenate([res.results[i]["out"] for i in range(NCORES)], 0)
    return y.reshape(B, N, L).astype(np.float32), res


def kernel(**inputs):
    return run(inputs)[0]


# revision 19
# speedup vs baseline: 1.2325x; 1.2325x over previous
"""AdaptiveTrendExtractor Trainium2 kernel (8-core data parallel).

Math (per row r of x reshaped to (B*N, L)):
  f_s = conv1d(x_r, w_s, 'same')            s in 4 scales (3,5,7,9), NO bias
  (softmax/entropy are invariant to the conv bias, so biases are folded
   into the final output as dot(weights, b) instead)
  Z_s   = sum_l exp(f_s)                     (no max-subtraction; |f| small)
  ent_s = log Z_s - (sum_l f_s*exp(f_s))/Z_s
  h     = relu((-ent) @ (-W1) + b1)          (we compute negent = -ent)
  wts   = softmax(h @ W2 + b2)
  out_r = sum_s wts_s * f_s + dot(wts, b)
"""

import numpy as np
import ml_dtypes

import concourse.bass as bass
import concourse.mybir as mybir
import concourse.tile as tile_mod
from concourse.tile import TileContext
from concourse.bass_utils import run_bass_kernel_spmd

F32 = mybir.dt.float32
BF16 = mybir.dt.bfloat16
AX = mybir.AxisListType
OP = mybir.AluOpType
AF = mybir.ActivationFunctionType

B, N, L = 64, 321, 720
NCORES = 8
R = B * N // NCORES          # 2568 rows per core
P = 128                      # partitions / rows per tile
NT = (R + P - 1) // P        # 21 tiles (last overlaps)
NC_CHUNK = 6                 # L chunks of 120 outputs each
CW = 120                     # outputs per chunk
PAD = 4                      # max k//2
SCALES = [3, 5, 7, 9]
NS = len(SCALES)
BANDW = NS * CW              # 480

# ---------------------------------------------------------------------------
# Patch: this walrus build rejects >1 sem wait on the TileContext final Drain
# (TPB_CTRL "Too many sync wait commands"); split waits over several drains.
_ScopedClock = tile_mod.ScopedClock


def _patched_dab(self, tick_clock, wait_clock):
    import bass_rust as _br

    nc = self.nc
    drain_inst = nc.sync.drain()
    wait_clock.add_sem_waits(
        drain_inst.ins, _ScopedClock({None: tick_clock.global_clock})
    )
    waits = list(drain_inst.ins.sync_info.on_wait)
    if len(waits) > 1:
        si = drain_inst.ins.sync_info
        si.on_wait = waits[:1]
        drain_inst.ins.sync_info = si
        for w in waits[1:]:
            d2 = nc.sync.drain()
            d2.ins.sync_info = _br.SyncInfo(on_wait=[w], on_update=[])
    nc.all_engine_barrier()
    popped = nc._tile_sem_poison_stack.pop()
    assert popped is self._sem_poison
    nc.clear_and_free_semaphores(list(self.sems.allocated().values()))
    nc.all_engine_barrier()


TileContext._drain_and_barrier = _patched_dab


def _split_excess_waits(nc, maxw=1):
    """walrus in this env rejects >maxw sem-waits on one instruction;
    hoist excess waits onto same-engine NoOps inserted just before."""
    import bass_rust as _br

    fn = nc.m.functions[0]
    # (block_idx, pos, engine, [waits]) for every violation
    plans = []
    for bi, blk in enumerate(fn.blocks):
        for pi, ins in enumerate(blk.instructions):
            si = ins.sync_info
            if si is None or not si.on_wait:
                continue
            waits = list(si.on_wait)
            if len(waits) > maxw:
                plans.append((bi, pi, ins, waits))
    if not plans:
        return
    # create nops (they get appended to the current bb; we'll move them)
    nop_map = {}
    created = []
    for bi, pi, ins, waits in plans:
        eng = nc.engines[ins.engine]
        nops = []
        for w in waits[:-maxw]:
            n = eng.nop()
            n.ins.sync_info = _br.SyncInfo(on_wait=[w], on_update=[])
            nops.append(n.ins)
            created.append(n.ins)
        si = ins.sync_info
        si.on_wait = waits[-maxw:]
        ins.sync_info = si
        nop_map[ins.name] = nops
    created_names = {n.name for n in created}
    for blk in fn.blocks:
        newl = []
        for ins in blk.instructions:
            if ins.name in created_names:
                continue  # remove from wherever engine.nop() appended it
            if ins.name in nop_map:
                newl.extend(nop_map[ins.name])
            newl.append(ins)
        blk.instructions = newl
# ---------------------------------------------------------------------------


def build_nc():
    nc = bass.Bass()
    x = nc.declare_dram_parameter("x", [R, L], F32, isOutput=False)
    bands = nc.declare_dram_parameter("bands", [P, BANDW], BF16, isOutput=False)
    identb = nc.declare_dram_parameter("identb", [P, P], BF16, isOutput=False)
    identf = nc.declare_dram_parameter("identf", [P, P], F32, isOutput=False)
    w1aug = nc.declare_dram_parameter("w1aug", [5, 32], F32, isOutput=False)
    w2aug = nc.declare_dram_parameter("w2aug", [33, 4], F32, isOutput=False)
    bvec = nc.declare_dram_parameter("bvec", [P, 4], F32, isOutput=False)
    y = nc.declare_dram_parameter("out", [R, L], F32, isOutput=True)

    with TileContext(nc) as tc:
        with (
            tc.tile_pool(name="const", bufs=1) as constp,
            tc.tile_pool(name="xin", bufs=4) as xpool,
            tc.tile_pool(name="xbf", bufs=4) as xbpool,
            tc.tile_pool(name="xts", bufs=3) as xtpool,
            tc.tile_pool(name="ebuf", bufs=4) as epool,
            tc.tile_pool(name="fbuf", bufs=4) as fbpool,
            tc.tile_pool(name="scr", bufs=3) as scrpool,
            tc.tile_pool(name="accp", bufs=6) as accpool,
            tc.tile_pool(name="youtp", bufs=3) as ypool,
            tc.tile_pool(name="small", bufs=8) as small,
            tc.tile_pool(name="fps", bufs=1, space="PSUM") as fpool,
            tc.tile_pool(name="mps", bufs=2, space="PSUM") as mpool,
        ):
            bands_t = constp.tile([P, BANDW], BF16)
            nc.sync.dma_start(out=bands_t[:], in_=bands[:])
            identb_t = constp.tile([P, P], BF16)
            nc.sync.dma_start(out=identb_t[:], in_=identb[:])
            identf_t = constp.tile([P, P], F32)
            nc.sync.dma_start(out=identf_t[:], in_=identf[:])
            w1_t = constp.tile([5, 32], F32)
            nc.sync.dma_start(out=w1_t[:], in_=w1aug[:])
            w2_t = constp.tile([33, 4], F32)
            nc.sync.dma_start(out=w2_t[:], in_=w2aug[:])
            bvec_t = constp.tile([P, 4], F32)
            nc.sync.dma_start(out=bvec_t[:], in_=bvec[:])

            def stage_a(t):
                r0 = min(P * t, R - P)
                st = {"r0": r0}
                # -- load + pad + cast -----------------------------------
                xp = xpool.tile([P, L + 2 * PAD], F32)
                nc.sync.dma_start(out=xp[:, PAD : PAD + L], in_=x[r0 : r0 + P, :])
                nc.gpsimd.memset(xp[:, 0:PAD], 0.0)
                nc.gpsimd.memset(xp[:, PAD + L :], 0.0)
                xb = xbpool.tile([P, L + 2 * PAD], BF16)
                nc.gpsimd.tensor_copy(xb[:], xp[:])

                # -- transpose 6 chunks to PSUM, evacuate to SBUF --------
                xtp = mpool.tile([P, NC_CHUNK * P], BF16, tag="ps")
                for c in range(NC_CHUNK):
                    nc.tensor.transpose(
                        xtp[:, c * P : (c + 1) * P],
                        xb[:, c * CW : c * CW + P],
                        identb_t[:],
                    )
                xt = xtpool.tile([P, NC_CHUNK * P], BF16)
                nc.vector.tensor_copy(xt[:], xtp[:])

                # -- conv: banded matmuls, all 4 scales per chunk --------
                f = fpool.tile([P, NC_CHUNK * 512], F32)
                for c in range(NC_CHUNK):
                    nc.tensor.matmul(
                        f[:, c * 512 : c * 512 + BANDW],
                        lhsT=xt[:, c * P : (c + 1) * P],
                        rhs=bands_t[:],
                        start=True,
                        stop=True,
                    )
                f_cx = f.rearrange("p (c x) -> p c x", x=512)
                f_scj = f_cx[:, :, 0:BANDW].rearrange("p c (s j) -> p s c j", j=CW)

                # -- evacuate f to SBUF bf16 (the ONLY PSUM reader, so
                #    next tile's conv can start after just this ~2.9us) --
                fb = fbpool.tile([P, NS * L], BF16)
                fb_scj = fb.rearrange("p (s c j) -> p s c j", c=NC_CHUNK, j=CW)
                nc.scalar.activation(fb_scj, f_scj, AF.Copy)
                # -- exp per scale from fb; accum_out gives Z for free ---
                e = epool.tile([P, NS * L], BF16)
                Zt = small.tile([P, 4], F32)
                for s in range(NS):
                    nc.scalar.activation(
                        e[:, s * L : (s + 1) * L],
                        fb[:, s * L : (s + 1) * L],
                        AF.Exp,
                        accum_out=Zt[:, s : s + 1],
                    )
                st.update(e=e, fb=fb, Zt=Zt)
                return st

            def stage_b(st):
                r0, e, fb, Zt = st["r0"], st["e"], st["fb"], st["Zt"]
                lnZ = small.tile([P, 4], F32)
                nc.scalar.activation(lnZ[:], Zt[:], AF.Ln)
                rZ = small.tile([P, 4], F32)
                nc.vector.reciprocal(rZ[:], Zt[:])

                # -- sum f*e per scale (fused mult+reduce) ---------------
                sfe = small.tile([P, 4], F32)
                scr = scrpool.tile([P, L], BF16)
                for s in range(NS):
                    nc.vector.scalar_tensor_tensor(
                        out=scr[:],
                        in0=e[:, s * L : (s + 1) * L],
                        scalar=1.0,
                        in1=fb[:, s * L : (s + 1) * L],
                        op0=OP.mult,
                        op1=OP.mult,
                        accum_out=sfe[:, s : s + 1],
                    )

                # -- negent = sfe/Z - lnZ  (col 4 = ones for bias aug) ---
                ne = small.tile([P, 4], F32)
                nc.gpsimd.tensor_tensor(ne[:], sfe[:], rZ[:], OP.mult)
                ne2 = small.tile([P, 5], F32)
                nc.gpsimd.tensor_tensor(ne2[:, 0:4], ne[:], lnZ[:], OP.subtract)
                nc.gpsimd.memset(ne2[:, 4:5], 1.0)

                # -- MLP -------------------------------------------------
                entTp = mpool.tile([5, P], F32, tag="ps")
                nc.tensor.transpose(entTp[:], ne2[:], identf_t[:])
                entT = small.tile([5, P], F32)
                nc.vector.tensor_copy(entT[:], entTp[:])
                hp = mpool.tile([P, 32], F32, tag="ps")
                nc.tensor.matmul(hp[:], lhsT=entT[:], rhs=w1_t[:], start=True, stop=True)
                h = small.tile([P, 33], F32)
                nc.vector.tensor_scalar_max(h[:, 0:32], hp[:], 0.0)
                nc.gpsimd.memset(h[:, 32:33], 1.0)
                hTp = mpool.tile([33, P], F32, tag="ps")
                nc.tensor.transpose(hTp[:], h[:], identf_t[:])
                hT = small.tile([33, P], F32)
                nc.vector.tensor_copy(hT[:], hTp[:])
                lgp = mpool.tile([P, 4], F32, tag="ps")
                nc.tensor.matmul(lgp[:], lhsT=hT[:], rhs=w2_t[:], start=True, stop=True)

                elog = small.tile([P, 4], F32)
                Z4 = small.tile([P, 1], F32)
                nc.scalar.activation(elog[:], lgp[:], AF.Exp, accum_out=Z4[:])
                rZ4 = small.tile([P, 1], F32)
                nc.vector.reciprocal(rZ4[:], Z4[:])
                wts = small.tile([P, 4], F32)
                nc.gpsimd.tensor_scalar(wts[:], elog[:], rZ4[:], None, OP.mult)
                wscr = small.tile([P, 4], F32)
                bdot = small.tile([P, 1], F32)
                nc.vector.scalar_tensor_tensor(
                    out=wscr[:],
                    in0=wts[:],
                    scalar=1.0,
                    in1=bvec_t[:],
                    op0=OP.mult,
                    op1=OP.mult,
                    accum_out=bdot[:],
                )

                # -- trend = sum_s wts_s * f_s + bdot --------------------
                acc0 = accpool.tile([P, L], BF16, tag="acc")
                acc1 = accpool.tile([P, L], BF16, tag="acc")
                acc2 = accpool.tile([P, L], BF16, tag="acc")
                yt = ypool.tile([P, L], F32)
                nc.vector.tensor_scalar(
                    acc0[:], fb[:, 0:L], wts[:, 0:1], bdot[:], OP.mult, OP.add
                )
                nc.vector.scalar_tensor_tensor(
                    acc1[:], fb[:, L : 2 * L], wts[:, 1:2], acc0[:], OP.mult, OP.add
                )
                nc.vector.scalar_tensor_tensor(
                    acc2[:], fb[:, 2 * L : 3 * L], wts[:, 2:3], acc1[:], OP.mult, OP.add
                )
                nc.vector.scalar_tensor_tensor(
                    yt[:], fb[:, 3 * L : 4 * L], wts[:, 3:4], acc2[:], OP.mult, OP.add
                )
                nc.sync.dma_start(out=y[r0 : r0 + P, :], in_=yt[:])

            pend = {}
            for t in range(NT + 2):
                if t < NT:
                    pend[t] = stage_a(t)
                if t >= 2:
                    stage_b(pend.pop(t - 2))
    _split_excess_waits(nc)
    return nc


_NC = None


def _get_nc():
    global _NC
    if _NC is None:
        _NC = build_nc()
    return _NC


def _host_consts(cw, cb, W1, b1, W2, b2):
    bands = np.zeros((P, BANDW), np.float32)
    for s, (k, w) in enumerate(zip(SCALES, cw)):
        w = np.asarray(w, np.float32).reshape(-1)
        for lp in range(CW):
            for j in range(k):
                kidx = lp + j + PAD - k // 2
                bands[kidx, s * CW + lp] = w[j]
    consts = {
        "bands": bands.astype(ml_dtypes.bfloat16),
        "identb": np.eye(P, dtype=ml_dtypes.bfloat16),
        "identf": np.eye(P, dtype=np.float32),
        "w1aug": np.concatenate(
            [-np.asarray(W1, np.float32), np.asarray(b1, np.float32)[None, :]], 0
        ),
        "w2aug": np.concatenate(
            [np.asarray(W2, np.float32), np.asarray(b2, np.float32)[None, :]], 0
        ),
        "bvec": np.tile(
            np.asarray(cb, np.float32).reshape(1, 4), (P, 1)
        ).astype(np.float32),
    }
    return consts


def run(inputs, **spmd_kwargs):
    nc = _get_nc()
    x = np.asarray(inputs["x"], np.float32).reshape(B * N, L)
    consts = _host_consts(
        [inputs[f"cw{i}"] for i in range(4)],
        [np.asarray(inputs[f"cb{i}"], np.float32).reshape(()) for i in range(4)],
        inputs["W1"],
        inputs["b1"],
        inputs["W2"],
        inputs["b2"],
    )
    in_maps = []
    for i in range(NCORES):
        m = {"x": np.ascontiguousarray(x[i * R : (i + 1) * R])}
        m.update(consts)
        in_maps.append(m)
    res = run_bass_kernel_spmd(nc, in_maps, core_ids=list(range(NCORES)), **spmd_kwargs)
    y = np.concatenate([res.results[i]["out"] for i in range(NCORES)], 0)
    return y.reshape(B, N, L).astype(np.float32), res


def kernel(**inputs):
    return run(inputs)[0]
